# revision 22
# baseline (speedup 1.0000x reference)
"""CRF Viterbi decode (forward max-plus scan + backtrace + one-hot) on 8
Trainium2 NeuronCores, data-parallel over the batch axis (32 batches/core).

v2 design (per core):
  Forward, blocked-replicated layout: partitions p = 32*jb + k hold batch k's
  alpha (full 128 classes) replicated over 4 class-blocks jb. Per step the
  max-plus inner loop is one elementwise ADD over [128, 4096] (split
  DVE/GPSIMD) + DVE max-reduces keeping the j_in axis. Alpha broadcast back to
  all partitions is done by small selection matmuls (zero-masked full-K
  lhsT constants; base-0 operands only -- PE tile_position row tiling is
  avoided entirely: positions >=64 fault at runtime and tiled matmuls run at
  reduced precision) that simultaneously add the emission potential via PSUM
  accumulation (pot matmul start=True first, allgathers accumulate stop=True).
  Masked batches keep alpha via copy_predicated. No backpointers in the
  forward pass; alpha history [T, 32, C] streams to DRAM (ACT copy + chunked
  DMA). When <=16 batches remain active on every core the program re-blocks
  to 8 class-blocks of 16 j_in (half the per-step work).

  Backtrace recomputes the winning backpointer per step for all 32 batches:
  cand[k,:] = alpha_hist[t-1][k,:] + T[:, tag_k] via the one-hot row (stored
  directly in the output staging ring by the previous step's is_equal) ->
  PE transpose -> matmul column-gather + hist selection matmul accumulated in
  PSUM, then DVE max/max_index; masked steps keep the tag via
  copy_predicated.
"""

import sys

sys.path.insert(0, "/opt/trn_rl_repo")

import numpy as np

B, T, C = 256, 1024, 128
NCORES = 8
BLOC = B // NCORES  # 32
CT = 64
NCH = T // CT

_prog_cache = {}
_consts_cache = None


def _assign_lanes(seq_lens: np.ndarray) -> np.ndarray:
    """Snake-deal batches (sorted by seq_len desc) to cores -> [NCORES, BLOC]."""
    order = np.argsort(-seq_lens, kind="stable")
    lanes = [[] for _ in range(NCORES)]
    for i, b in enumerate(order):
        r, k = divmod(i, NCORES)
        c = k if r % 2 == 0 else NCORES - 1 - k
        lanes[c].append(int(b))
    return np.array(lanes, dtype=np.int64)


def _modes(seq_lens, lanes):
    """Per-chunk mode (32 or 16 active-lane blocking), max over cores."""
    L = seq_lens[lanes]
    modes = []
    for c in range(NCH):
        t0 = c * CT
        act = max(int((row > t0).sum()) for row in L)
        modes.append(32 if act > 16 else 16)
    return tuple(modes)


def _build_consts():
    """Shared (core-independent) host constant arrays."""
    ident = np.eye(C, dtype=np.float32)
    esel2 = np.zeros((128, 128), np.float32)
    for p in range(128):
        esel2[p, np.arange(p % 32, 128, 32)] = 1.0
    e16 = np.zeros((128, 128), np.float32)
    for p in range(128):
        e16[p, np.arange(p % 16, 128, 16)] = 1.0
    # zero-masked block selectors: eselJ32[:, jb] rows only for block jb
    eJ32 = np.zeros((4, 128, 128), np.float32)
    for jb in range(4):
        eJ32[jb, 32 * jb:32 * (jb + 1), :] = esel2[32 * jb:32 * (jb + 1), :]
    eJ16 = np.zeros((8, 128, 128), np.float32)
    for bb in range(8):
        eJ16[bb, 16 * bb:16 * (bb + 1), :] = e16[16 * bb:16 * (bb + 1), :]
    iotaf = np.tile(np.arange(C, dtype=np.float32), (BLOC, 1))
    return {"ident": ident, "esel2": esel2, "e16": e16,
            "eJ32": np.ascontiguousarray(eJ32.reshape(4 * 128, 128)),
            "eJ16": np.ascontiguousarray(eJ16.reshape(8 * 128, 128)),
            "iotaf": np.ascontiguousarray(iotaf)}


def _build_treps(transitions):
    tt = np.ascontiguousarray(transitions.T).astype(np.float32)  # tt[j,i]=T[i,j]
    trep32 = np.empty((128, 32 * C), np.float32)
    for p in range(128):
        jb = p // 32
        trep32[p] = tt[32 * jb:32 * (jb + 1), :].reshape(-1)
    trep16 = np.empty((128, 16 * C), np.float32)
    for p in range(128):
        jb = p // 16
        trep16[p] = tt[16 * jb:16 * (jb + 1), :].reshape(-1)
    return tt, trep32, trep16


def build_program(modes, trn="TRN2", num_devices=NCORES, phase="all"):
    import concourse.bass as bass
    import concourse.bacc as bacc
    import concourse.mybir as mybir
    from concourse import tile

    f32 = mybir.dt.float32
    u16 = mybir.dt.uint16
    i8 = mybir.dt.int8
    Alu = mybir.AluOpType
    AX = mybir.AxisListType.X

    nc = bacc.Bacc(trn, target_bir_lowering=False, debug=False,
                   num_devices=num_devices)

    TC = T * C
    x_d = nc.dram_tensor("x", [BLOC, TC], f32, kind="ExternalInput").ap()
    trep32_d = nc.dram_tensor("trep32", [128, 32 * C], f32, kind="ExternalInput").ap()
    trep16_d = nc.dram_tensor("trep16", [128, 16 * C], f32, kind="ExternalInput").ap()
    tt_d = nc.dram_tensor("tt", [C, C], f32, kind="ExternalInput").ap()
    ident_d = nc.dram_tensor("ident", [C, C], f32, kind="ExternalInput").ap()
    esel2_d = nc.dram_tensor("esel2", [128, 128], f32, kind="ExternalInput").ap()
    e16_d = nc.dram_tensor("e16", [128, 128], f32, kind="ExternalInput").ap()
    eJ32_d = nc.dram_tensor("eJ32", [4 * 128, 128], f32, kind="ExternalInput").ap()
    eJ16_d = nc.dram_tensor("eJ16", [8 * 128, 128], f32, kind="ExternalInput").ap()
    iotaf_d = nc.dram_tensor("iotaf", [BLOC, C], f32, kind="ExternalInput").ap()
    mk32_d = nc.dram_tensor("mk32", [128, T], i8, kind="ExternalInput").ap()
    mk16_d = nc.dram_tensor("mk16", [128, T], i8, kind="ExternalInput").ap()
    ivm_d = nc.dram_tensor("ivm", [BLOC, T], i8, kind="ExternalInput").ap()
    hist_d = nc.dram_tensor("hist", [BLOC, TC], f32)  # internal scratch
    out_d = nc.dram_tensor("out", [BLOC, TC], f32, kind="ExternalOutput").ap()

    # persistent SBUF.  eJ32/eJ16 selectors are stored as free-axis
    # concatenations [128, nsel*128] so every matmul lhsT slice is base-0.
    trep32_sb = nc.alloc_sbuf_tensor("trep32_sb", [128, 32 * C], f32).ap()
    trep16_sb = nc.alloc_sbuf_tensor("trep16_sb", [128, 16 * C], f32).ap()
    tt_sb = nc.alloc_sbuf_tensor("tt_sb", [C, C], f32).ap()
    ident_sb = nc.alloc_sbuf_tensor("ident_sb", [C, C], f32).ap()
    esel2_sb = nc.alloc_sbuf_tensor("esel2_sb", [128, 128], f32).ap()
    e16_sb = nc.alloc_sbuf_tensor("e16_sb", [128, 128], f32).ap()
    eJ32_sb = nc.alloc_sbuf_tensor("eJ32_sb", [128, 4 * 128], f32).ap()
    eJ16_sb = nc.alloc_sbuf_tensor("eJ16_sb", [128, 8 * 128], f32).ap()
    iotaf_sb = nc.alloc_sbuf_tensor("iotaf_sb", [BLOC, C], f32).ap()
    mk32_sb = nc.alloc_sbuf_tensor("mk32_sb", [128, T], i8)
    mk16_sb = nc.alloc_sbuf_tensor("mk16_sb", [128, T], i8)
    ivm_sb = nc.alloc_sbuf_tensor("ivm_sb", [BLOC, T], i8)
    alpha = nc.alloc_sbuf_tensor("alpha", [128, C], f32).ap()
    alpha_fin = nc.alloc_sbuf_tensor("alpha_fin", [BLOC, C], f32).ap()
    m_sb = nc.alloc_sbuf_tensor("m_sb", [128, 32], f32).ap()
    sc_t = nc.alloc_sbuf_tensor("sc_sb", [128, 32 * C], f32)
    sc_sb = sc_t.ap()
    ring = [nc.alloc_sbuf_tensor(f"ring{p}", [BLOC, CT * C], f32).ap()
            for p in (0, 1)]
    hr = [nc.alloc_sbuf_tensor(f"hr{p}", [BLOC, CT * C], f32).ap()
          for p in (0, 1)]
    # backtrace tag slots alias the forward scratch (disjoint lifetimes)
    tg8 = sc_t.ap().bitcast(u16)
    ohTs = nc.alloc_sbuf_tensor("ohTs", [C, 32], f32).ap()
    mx32 = nc.alloc_sbuf_tensor("mx32", [BLOC, 1], f32)
    tagf = nc.alloc_sbuf_tensor("tagf", [BLOC, T], f32).ap()

    def bcast(ap, dim, n):
        a = ap[tuple(slice(None) for _ in ap.shape)]
        a.ap.insert(dim, [0, n])
        return a

    def mask_col(msk, t, n):
        return bass.AP(msk, t, [[T, n], [0, C]])

    mx8 = bass.AP(mx32, 0, [[1, BLOC], [0, 8]])

    with tile.TileContext(nc) as tc:
        with (
            tc.tile_pool(name="psA", bufs=2, space="PSUM") as psA,
            tc.tile_pool(name="psT", bufs=2, space="PSUM") as psT,
            tc.tile_pool(name="psC", bufs=2, space="PSUM") as psC,
        ):
            # one-time loads
            for dst, src in [(trep32_sb, trep32_d), (trep16_sb, trep16_d),
                             (tt_sb, tt_d), (ident_sb, ident_d),
                             (esel2_sb, esel2_d), (e16_sb, e16_d),
                             (iotaf_sb, iotaf_d), (mk32_sb.ap(), mk32_d),
                             (mk16_sb.ap(), mk16_d), (ivm_sb.ap(), ivm_d)]:
                nc.sync.dma_start(out=dst, in_=src)
            for jb in range(4):
                nc.sync.dma_start(out=eJ32_sb[:, 128 * jb:128 * (jb + 1)],
                                  in_=eJ32_d[128 * jb:128 * (jb + 1), :])
            for bb in range(8):
                nc.sync.dma_start(out=eJ16_sb[:, 128 * bb:128 * (bb + 1)],
                                  in_=eJ16_d[128 * bb:128 * (bb + 1), :])
            for jb in range(4):
                nc.sync.dma_start(out=alpha[32 * jb:32 * (jb + 1), :],
                                  in_=x_d[:, 0:C])
            nc.sync.dma_start(out=ring[0], in_=x_d[:, 0:CT * C])

            # ---------------- forward ----------------
            def fwd_step(c, s, mode, pr):
                t = c * CT + s
                ps = psA.tile([128, C], f32, tag="ps")
                # 16-mode pot needs rows 16..31 zero-masked (eJ16 block 0)
                potsel = esel2_sb if mode == 32 else eJ16_sb
                nc.tensor.matmul(ps[:], potsel[0:32, 0:128],
                                 ring[pr][:, s * C:(s + 1) * C],
                                 start=True, stop=False, skip_group_check=True)
                if mode == 32:
                    groups = ((0, 2, "v"), (2, 12, "g"), (12, 22, "g"),
                              (22, 32, "g"))
                    waves = ((0, 22), (22, 32))
                    trep, msk = trep32_sb, mk32_sb
                    nsel, selw, sel = 4, 32, eJ32_sb
                else:
                    groups = ((0, 1, "v"), (1, 8, "g"), (8, 16, "g"))
                    waves = ((0, 8), (8, 16))
                    trep, msk = trep16_sb, mk16_sb
                    nsel, selw, sel = 8, 16, eJ16_sb
                for (lo, hi, eng) in groups:
                    e = nc.vector if eng == "v" else nc.gpsimd
                    e.tensor_tensor(
                        out=sc_sb[:, lo * C:hi * C].rearrange(
                            "p (j i) -> p j i", i=C),
                        in0=bcast(alpha, 1, hi - lo),
                        in1=trep[:, lo * C:hi * C].rearrange(
                            "p (j i) -> p j i", i=C),
                        op=Alu.add)
                for (lo, hi, _) in groups:
                    nc.vector.tensor_reduce(
                        out=m_sb[:, lo:hi],
                        in_=sc_sb[:, lo * C:hi * C].rearrange(
                            "p (j i) -> p j i", i=C),
                        axis=AX, op=Alu.max)
                # allgather waves: zero-masked full-K selection matmuls
                for (lo, hi) in waves:
                    for b in range(nsel):
                        nc.tensor.matmul(
                            ps[:, selw * b + lo:selw * b + hi],
                            sel[:, 128 * b:128 * (b + 1)],
                            m_sb[:, lo:hi],
                            start=False, stop=True, skip_group_check=True)
                nc.vector.copy_predicated(out=alpha,
                                          mask=mask_col(msk, t, 128),
                                          data=ps[:])

            for c in range(NCH):
                mode = modes[c]
                pr = c % 2
                if c + 1 < NCH:
                    nc.sync.dma_start(
                        out=ring[(c + 1) % 2],
                        in_=x_d[:, (c + 1) * CT * C:(c + 2) * CT * C])
                if c > 0 and modes[c - 1] == 32 and mode == 16:
                    # stash final alpha (batches 16..31 die here), re-block
                    nc.scalar.copy(out=alpha_fin, in_=alpha[0:32, :])
                    pst = psA.tile([128, C], f32, tag="ps")
                    nc.tensor.matmul(pst[:], eJ16_sb[0:32, 0:128],
                                     alpha[0:32, :], start=True, stop=True)
                    nc.vector.tensor_copy(out=alpha, in_=pst[:])
                for s in range(CT):
                    if not (c == 0 and s == 0):
                        fwd_step(c, s, mode, pr)
                    nc.scalar.copy(out=hr[pr][:, s * C:(s + 1) * C],
                                   in_=alpha[0:32, :])
                nc.sync.dma_start(
                    out=hist_d.ap()[:, c * CT * C:(c + 1) * CT * C],
                    in_=hr[pr])

            # ---------------- last tag ----------------
            if 16 in modes:
                nc.scalar.copy(out=alpha_fin[0:16, :], in_=alpha[0:16, :])
            else:
                nc.scalar.copy(out=alpha_fin, in_=alpha[0:32, :])
            if phase != "fwd":
                nc.vector.tensor_reduce(out=mx32.ap(), in_=alpha_fin,
                                        axis=AX, op=Alu.max)
                nc.vector.max_index(out=tg8[0:BLOC, 8 * (T - 1):8 * T],
                                    in_max=mx8, in_values=alpha_fin)
                nc.gpsimd.tensor_copy(
                    out=tagf[:, T - 1:T],
                    in_=tg8[0:BLOC, 8 * (T - 1):8 * (T - 1) + 1])
                nc.gpsimd.tensor_scalar(
                    out=hr[(NCH - 1) % 2][:, (CT - 1) * C:CT * C],
                    in0=iotaf_sb, scalar1=tagf[:, T - 1:T],
                    scalar2=None, op0=Alu.is_equal)

                # ---------------- backtrace ----------------
                def bt_load(cc, pr2):
                    lo = max(cc * CT - 1, 0) * C
                    nc.sync.dma_start(out=ring[pr2],
                                      in_=hist_d.ap()[:, lo:lo + CT * C])

                bt_load(NCH - 1, (NCH - 1) % 2)
                for cc in range(NCH - 1, -1, -1):
                    pr = cc % 2
                    if cc > 0:
                        bt_load(cc - 1, (cc - 1) % 2)
                    for s in range(CT - 1, -1, -1):
                        u = cc * CT + s
                        if u == 0:
                            continue
                        col = s if cc > 0 else s - 1
                        tp = psT.tile([C, 32], f32, tag="tp")
                        nc.tensor.transpose(
                            tp[:], hr[pr][:, s * C:(s + 1) * C],
                            ident_sb[0:32, 0:32])
                        nc.scalar.copy(out=ohTs, in_=tp[:])
                        cand = psC.tile([BLOC, C], f32, tag="cand")
                        nc.tensor.matmul(cand[:], ohTs, tt_sb,
                                         start=True, stop=False,
                                         skip_group_check=True)
                        nc.tensor.matmul(cand[:], esel2_sb[0:32, 0:32],
                                         ring[pr][:, col * C:(col + 1) * C],
                                         start=False, stop=True,
                                         skip_group_check=True)
                        nc.vector.tensor_reduce(out=mx32.ap(), in_=cand[:],
                                                axis=AX, op=Alu.max)
                        nc.vector.max_index(
                            out=tg8[0:BLOC, 8 * (u - 1):8 * u],
                            in_max=mx8, in_values=cand[:])
                        nc.vector.copy_predicated(
                            out=tg8[0:BLOC, 8 * (u - 1):8 * (u - 1) + 1],
                            mask=ivm_sb.ap()[:, u:u + 1],
                            data=tg8[0:BLOC, 8 * u:8 * u + 1])
                        nc.gpsimd.tensor_copy(
                            out=tagf[:, u - 1:u],
                            in_=tg8[0:BLOC, 8 * (u - 1):8 * (u - 1) + 1])
                        ecol = s - 1 if s > 0 else CT - 1
                        epr = pr if s > 0 else 1 - pr
                        nc.gpsimd.tensor_scalar(
                            out=hr[epr][:, ecol * C:(ecol + 1) * C],
                            in0=iotaf_sb, scalar1=tagf[:, u - 1:u],
                            scalar2=None, op0=Alu.is_equal)
                    nc.sync.dma_start(
                        out=out_d[:, cc * CT * C:(cc + 1) * CT * C],
                        in_=hr[pr])

    nc.compile()
    return nc


def _host_inputs(x, transitions, seq_lens, lanes):
    global _consts_cache
    if _consts_cache is None:
        _consts_cache = _build_consts()
    cst = _consts_cache
    tt, trep32, trep16 = _build_treps(transitions)
    tgrid = np.arange(T)[None, :]
    in_maps = []
    for c in range(lanes.shape[0]):
        lx = x[lanes[c]].reshape(BLOC, T * C).astype(np.float32)
        L = seq_lens[lanes[c]].astype(np.int64)
        mk = (tgrid < L[:, None]).astype(np.int8)          # [32, T]
        mk32 = np.ascontiguousarray(np.tile(mk, (4, 1)))    # [128, T]
        mk16 = np.ascontiguousarray(np.tile(mk[:16], (8, 1)))
        ivm = np.ascontiguousarray((tgrid >= L[:, None]).astype(np.int8))
        in_maps.append({
            "x": np.ascontiguousarray(lx),
            "trep32": trep32, "trep16": trep16, "tt": tt,
            "ident": cst["ident"], "esel2": cst["esel2"], "e16": cst["e16"],
            "eJ32": cst["eJ32"], "eJ16": cst["eJ16"], "iotaf": cst["iotaf"],
            "mk32": mk32, "mk16": mk16, "ivm": ivm,
        })
    return in_maps


TRACE = False
LAST_RESULT = None


def kernel(x, transitions, seq_lens):
    global LAST_RESULT
    from concourse.bass_utils import run_bass_kernel_spmd

    x = np.asarray(x, dtype=np.float32)
    transitions = np.asarray(transitions, dtype=np.float32)
    seq_lens = np.asarray(seq_lens)
    lanes = _assign_lanes(seq_lens)
    modes = _modes(seq_lens, lanes)
    if modes not in _prog_cache:
        _prog_cache[modes] = build_program(modes)
    nc = _prog_cache[modes]
    in_maps = _host_inputs(x, transitions, seq_lens, lanes)
    res = run_bass_kernel_spmd(nc, in_maps, list(range(NCORES)), trace=TRACE)
    LAST_RESULT = res
    out = np.empty((B, T, C), np.float32)
    for c in range(NCORES):
        out[lanes[c]] = res.results[c]["out"].reshape(BLOC, T, C)
    return out


# revision 24
# speedup vs baseline: 1.4768x; 1.4768x over previous
"""CRF Viterbi decode (forward max-plus scan + backtrace + one-hot) on 8
Trainium2 NeuronCores, data-parallel over the batch axis (32 batches/core).

v2 design (per core):
  Forward, blocked-replicated layout: partitions p = 32*jb + k hold batch k's
  alpha (full 128 classes) replicated over 4 class-blocks jb. Per step the
  max-plus inner loop is one elementwise ADD over [128, 4096] (split
  DVE/GPSIMD) + DVE max-reduces keeping the j_in axis. Alpha broadcast back to
  all partitions is done by small selection matmuls (zero-masked full-K
  lhsT constants; base-0 operands only -- PE tile_position row tiling is
  avoided entirely: positions >=64 fault at runtime and tiled matmuls run at
  reduced precision) that simultaneously add the emission potential via PSUM
  accumulation (pot matmul start=True first, allgathers accumulate stop=True).
  Masked batches keep alpha via copy_predicated. No backpointers in the
  forward pass; alpha history [T, 32, C] streams to DRAM (ACT copy + chunked
  DMA). When <=16 batches remain active on every core the program re-blocks
  to 8 class-blocks of 16 j_in (half the per-step work).

  Backtrace recomputes the winning backpointer per step for all 32 batches:
  cand[k,:] = alpha_hist[t-1][k,:] + T[:, tag_k] via the one-hot row (stored
  directly in the output staging ring by the previous step's is_equal) ->
  PE transpose -> matmul column-gather + hist selection matmul accumulated in
  PSUM, then DVE max/max_index; masked steps keep the tag via
  copy_predicated.
"""

import sys

sys.path.insert(0, "/opt/trn_rl_repo")

import numpy as np

B, T, C = 256, 1024, 128
NCORES = 8
BLOC = B // NCORES  # 32
CT = 64
NCH = T // CT

_prog_cache = {}
_consts_cache = None


def _assign_lanes(seq_lens: np.ndarray) -> np.ndarray:
    """Snake-deal batches (sorted by seq_len desc) to cores -> [NCORES, BLOC]."""
    order = np.argsort(-seq_lens, kind="stable")
    lanes = [[] for _ in range(NCORES)]
    for i, b in enumerate(order):
        r, k = divmod(i, NCORES)
        c = k if r % 2 == 0 else NCORES - 1 - k
        lanes[c].append(int(b))
    return np.array(lanes, dtype=np.int64)


def _modes(seq_lens, lanes):
    """Per-chunk mode (32 or 16 active-lane blocking), max over cores."""
    L = seq_lens[lanes]
    modes = []
    for c in range(NCH):
        t0 = c * CT
        act = max(int((row > t0).sum()) for row in L)
        modes.append(32 if act > 16 else 16)
    return tuple(modes)


def _build_consts():
    """Shared (core-independent) host constant arrays."""
    ident = np.eye(C, dtype=np.float32)
    esel2 = np.zeros((128, 128), np.float32)
    for p in range(128):
        esel2[p, np.arange(p % 32, 128, 32)] = 1.0
    e16 = np.zeros((128, 128), np.float32)
    for p in range(128):
        e16[p, np.arange(p % 16, 128, 16)] = 1.0
    # zero-masked block selectors: eselJ32[:, jb] rows only for block jb
    eJ32 = np.zeros((4, 128, 128), np.float32)
    for jb in range(4):
        eJ32[jb, 32 * jb:32 * (jb + 1), :] = esel2[32 * jb:32 * (jb + 1), :]
    eJ16 = np.zeros((8, 128, 128), np.float32)
    for bb in range(8):
        eJ16[bb, 16 * bb:16 * (bb + 1), :] = e16[16 * bb:16 * (bb + 1), :]
    iotaf = np.tile(np.arange(C, dtype=np.float32), (BLOC, 1))
    return {"ident": ident, "esel2": esel2, "e16": e16,
            "eJ32": np.ascontiguousarray(eJ32.reshape(4 * 128, 128)),
            "eJ16": np.ascontiguousarray(eJ16.reshape(8 * 128, 128)),
            "iotaf": np.ascontiguousarray(iotaf)}


def _build_treps(transitions):
    tt = np.ascontiguousarray(transitions.T).astype(np.float32)  # tt[j,i]=T[i,j]
    trep32 = np.empty((128, 32 * C), np.float32)
    for p in range(128):
        jb = p // 32
        trep32[p] = tt[32 * jb:32 * (jb + 1), :].reshape(-1)
    trep16 = np.empty((128, 16 * C), np.float32)
    for p in range(128):
        jb = p // 16
        trep16[p] = tt[16 * jb:16 * (jb + 1), :].reshape(-1)
    return tt, trep32, trep16


def build_program(modes, trn="TRN2", num_devices=NCORES, phase="all"):
    import concourse.bass as bass
    import concourse.bacc as bacc
    import concourse.mybir as mybir
    from concourse import tile

    f32 = mybir.dt.float32
    u16 = mybir.dt.uint16
    i8 = mybir.dt.int8
    Alu = mybir.AluOpType
    AX = mybir.AxisListType.X

    nc = bacc.Bacc(trn, target_bir_lowering=False, debug=False,
                   num_devices=num_devices)

    TC = T * C
    x_d = nc.dram_tensor("x", [BLOC, TC], f32, kind="ExternalInput").ap()
    trep32_d = nc.dram_tensor("trep32", [128, 32 * C], f32, kind="ExternalInput").ap()
    trep16_d = nc.dram_tensor("trep16", [128, 16 * C], f32, kind="ExternalInput").ap()
    tt_d = nc.dram_tensor("tt", [C, C], f32, kind="ExternalInput").ap()
    ident_d = nc.dram_tensor("ident", [C, C], f32, kind="ExternalInput").ap()
    esel2_d = nc.dram_tensor("esel2", [128, 128], f32, kind="ExternalInput").ap()
    e16_d = nc.dram_tensor("e16", [128, 128], f32, kind="ExternalInput").ap()
    eJ32_d = nc.dram_tensor("eJ32", [4 * 128, 128], f32, kind="ExternalInput").ap()
    eJ16_d = nc.dram_tensor("eJ16", [8 * 128, 128], f32, kind="ExternalInput").ap()
    iotaf_d = nc.dram_tensor("iotaf", [BLOC, C], f32, kind="ExternalInput").ap()
    mk32_d = nc.dram_tensor("mk32", [128, T], i8, kind="ExternalInput").ap()
    mk16_d = nc.dram_tensor("mk16", [128, T], i8, kind="ExternalInput").ap()
    ivm_d = nc.dram_tensor("ivm", [BLOC, T], i8, kind="ExternalInput").ap()
    hist_d = nc.dram_tensor("hist", [BLOC, TC], f32)  # internal scratch
    out_d = nc.dram_tensor("out", [BLOC, TC], f32, kind="ExternalOutput").ap()

    # persistent SBUF.  eJ32/eJ16 selectors are stored as free-axis
    # concatenations [128, nsel*128] so every matmul lhsT slice is base-0.
    trep32_sb = nc.alloc_sbuf_tensor("trep32_sb", [128, 32 * C], f32).ap()
    trep16_sb = nc.alloc_sbuf_tensor("trep16_sb", [128, 16 * C], f32).ap()
    tt_sb = nc.alloc_sbuf_tensor("tt_sb", [C, C], f32).ap()
    ident_sb = nc.alloc_sbuf_tensor("ident_sb", [C, C], f32).ap()
    esel2_sb = nc.alloc_sbuf_tensor("esel2_sb", [128, 128], f32).ap()
    e16_sb = nc.alloc_sbuf_tensor("e16_sb", [128, 128], f32).ap()
    eJ32_sb = nc.alloc_sbuf_tensor("eJ32_sb", [128, 4 * 128], f32).ap()
    eJ16_sb = nc.alloc_sbuf_tensor("eJ16_sb", [128, 8 * 128], f32).ap()
    iotaf_sb = nc.alloc_sbuf_tensor("iotaf_sb", [BLOC, C], f32).ap()
    mk32_sb = nc.alloc_sbuf_tensor("mk32_sb", [128, T], i8)
    mk16_sb = nc.alloc_sbuf_tensor("mk16_sb", [128, T], i8)
    ivm_sb = nc.alloc_sbuf_tensor("ivm_sb", [BLOC, T], i8)
    alpha = nc.alloc_sbuf_tensor("alpha", [128, C], f32).ap()
    alpha_fin = nc.alloc_sbuf_tensor("alpha_fin", [BLOC, C], f32).ap()
    m_sb = nc.alloc_sbuf_tensor("m_sb", [128, 32], f32).ap()
    sc_t = nc.alloc_sbuf_tensor("sc_sb", [128, 32 * C], f32)
    sc_sb = sc_t.ap()
    ring = [nc.alloc_sbuf_tensor(f"ring{p}", [BLOC, CT * C], f32).ap()
            for p in (0, 1)]
    hr = [nc.alloc_sbuf_tensor(f"hr{p}", [BLOC, CT * C], f32).ap()
          for p in (0, 1)]
    # backtrace tag slots alias the forward scratch (disjoint lifetimes)
    tg8 = sc_t.ap().bitcast(u16)
    ohTs = nc.alloc_sbuf_tensor("ohTs", [C, 32], f32).ap()
    mx32 = nc.alloc_sbuf_tensor("mx32", [BLOC, 1], f32)
    tagf = nc.alloc_sbuf_tensor("tagf", [BLOC, T], f32).ap()
    ltf = nc.alloc_sbuf_tensor("ltf", [BLOC, 1], f32).ap()
    tgc = nc.alloc_sbuf_tensor("tgc", [BLOC, 1], f32).ap()
    candsb = nc.alloc_sbuf_tensor("candsb", [BLOC, C], f32).ap()

    def bcast(ap, dim, n):
        a = ap[tuple(slice(None) for _ in ap.shape)]
        a.ap.insert(dim, [0, n])
        return a

    def mask_col(msk, t, n):
        return bass.AP(msk, t, [[T, n], [0, C]])

    mx8 = bass.AP(mx32, 0, [[1, BLOC], [0, 8]])

    with tile.TileContext(nc) as tc:
        with (
            tc.tile_pool(name="psA", bufs=2, space="PSUM") as psA,
            tc.tile_pool(name="psT", bufs=2, space="PSUM") as psT,
            tc.tile_pool(name="psC", bufs=2, space="PSUM") as psC,
        ):
            # one-time loads
            for dst, src in [(trep32_sb, trep32_d), (trep16_sb, trep16_d),
                             (tt_sb, tt_d), (ident_sb, ident_d),
                             (esel2_sb, esel2_d), (e16_sb, e16_d),
                             (iotaf_sb, iotaf_d), (mk32_sb.ap(), mk32_d),
                             (mk16_sb.ap(), mk16_d), (ivm_sb.ap(), ivm_d)]:
                nc.sync.dma_start(out=dst, in_=src)
            for jb in range(4):
                nc.sync.dma_start(out=eJ32_sb[:, 128 * jb:128 * (jb + 1)],
                                  in_=eJ32_d[128 * jb:128 * (jb + 1), :])
            for bb in range(8):
                nc.sync.dma_start(out=eJ16_sb[:, 128 * bb:128 * (bb + 1)],
                                  in_=eJ16_d[128 * bb:128 * (bb + 1), :])
            for jb in range(4):
                nc.sync.dma_start(out=alpha[32 * jb:32 * (jb + 1), :],
                                  in_=x_d[:, 0:C])
            nc.sync.dma_start(out=ring[0], in_=x_d[:, 0:CT * C])

            # ---------------- forward ----------------
            def fwd_step(c, s, mode, pr):
                t = c * CT + s
                ps = psA.tile([128, C], f32, tag="ps")
                # 16-mode pot needs rows 16..31 zero-masked (eJ16 block 0)
                potsel = esel2_sb if mode == 32 else eJ16_sb
                nc.tensor.matmul(ps[:], potsel[0:32, 0:128],
                                 ring[pr][:, s * C:(s + 1) * C],
                                 start=True, stop=False, skip_group_check=True)
                if mode == 32:
                    groups = ((0, 20, "v"), (20, 26, "g"), (26, 32, "g"))
                    trep, msk = trep32_sb, mk32_sb
                    nsel, selw, sel = 4, 32, eJ32_sb
                else:
                    groups = ((0, 10, "v"), (10, 16, "g"))
                    trep, msk = trep16_sb, mk16_sb
                    nsel, selw, sel = 8, 16, eJ16_sb
                for (lo, hi, eng) in groups:
                    e = nc.vector if eng == "v" else nc.gpsimd
                    e.tensor_tensor(
                        out=sc_sb[:, lo * C:hi * C].rearrange(
                            "p (j i) -> p j i", i=C),
                        in0=bcast(alpha, 1, hi - lo),
                        in1=trep[:, lo * C:hi * C].rearrange(
                            "p (j i) -> p j i", i=C),
                        op=Alu.add)
                for (lo, hi, _) in groups:
                    nc.vector.tensor_reduce(
                        out=m_sb[:, lo:hi],
                        in_=sc_sb[:, lo * C:hi * C].rearrange(
                            "p (j i) -> p j i", i=C),
                        axis=AX, op=Alu.max)
                # allgather: one zero-masked selection matmul per block,
                # with the smallest legal K (partitions above the block's
                # rows are never read)
                for b in range(nsel):
                    kk = selw * (b + 1)
                    kk = 32 if kk <= 32 else (64 if kk <= 64 else 128)
                    nc.tensor.matmul(
                        ps[:, selw * b:selw * (b + 1)],
                        sel[0:kk, 128 * b:128 * (b + 1)],
                        m_sb[0:kk, 0:selw],
                        start=False, stop=True, skip_group_check=True)
                nc.vector.copy_predicated(out=alpha,
                                          mask=mask_col(msk, t, 128),
                                          data=ps[:])

            for c in range(NCH):
                mode = modes[c]
                pr = c % 2
                if c + 1 < NCH:
                    nc.sync.dma_start(
                        out=ring[(c + 1) % 2],
                        in_=x_d[:, (c + 1) * CT * C:(c + 2) * CT * C])
                if c > 0 and modes[c - 1] == 32 and mode == 16:
                    # stash final alpha (batches 16..31 die here), re-block
                    nc.scalar.copy(out=alpha_fin, in_=alpha[0:32, :])
                    pst = psA.tile([128, C], f32, tag="ps")
                    nc.tensor.matmul(pst[:], eJ16_sb[0:32, 0:128],
                                     alpha[0:32, :], start=True, stop=True)
                    nc.vector.tensor_copy(out=alpha, in_=pst[:])
                for s in range(CT):
                    if not (c == 0 and s == 0):
                        fwd_step(c, s, mode, pr)
                    nc.scalar.copy(out=hr[pr][:, s * C:(s + 1) * C],
                                   in_=alpha[0:32, :])
                nc.sync.dma_start(
                    out=hist_d.ap()[:, c * CT * C:(c + 1) * CT * C],
                    in_=hr[pr])

            # ---------------- last tag ----------------
            if 16 in modes:
                nc.scalar.copy(out=alpha_fin[0:16, :], in_=alpha[0:16, :])
            else:
                nc.scalar.copy(out=alpha_fin, in_=alpha[0:32, :])
            if phase != "fwd":
                nc.vector.tensor_reduce(out=mx32.ap(), in_=alpha_fin,
                                        axis=AX, op=Alu.max)
                nc.vector.max_index(out=tg8[0:BLOC, 8 * (T - 1):8 * T],
                                    in_max=mx8, in_values=alpha_fin)
                nc.vector.tensor_copy(
                    out=ltf, in_=tg8[0:BLOC, 8 * (T - 1):8 * (T - 1) + 1])
                # prefill all tags with last_tag: masked (dead) steps keep it,
                # so the per-step override copy becomes a single masked cast
                nc.scalar.activation(
                    out=tagf, in_=tagf,
                    func=mybir.ActivationFunctionType.Relu,
                    bias=ltf[:, 0:1], scale=0.0)
                nc.vector.tensor_scalar(
                    out=hr[(NCH - 1) % 2][:, (CT - 1) * C:CT * C],
                    in0=iotaf_sb, scalar1=tagf[:, T - 1:T],
                    scalar2=None, op0=Alu.is_equal)

                # ---------------- backtrace ----------------
                def bt_load(cc, pr2):
                    lo = max(cc * CT - 1, 0) * C
                    nc.sync.dma_start(out=ring[pr2],
                                      in_=hist_d.ap()[:, lo:lo + CT * C])

                bt_load(NCH - 1, (NCH - 1) % 2)
                for cc in range(NCH - 1, -1, -1):
                    pr = cc % 2
                    if cc > 0:
                        bt_load(cc - 1, (cc - 1) % 2)
                    for s in range(CT - 1, -1, -1):
                        u = cc * CT + s
                        if u == 0:
                            continue
                        col = s if cc > 0 else s - 1
                        tp = psT.tile([C, 32], f32, tag="tp")
                        nc.tensor.transpose(
                            tp[:], hr[pr][:, s * C:(s + 1) * C],
                            ident_sb[0:32, 0:32])
                        nc.scalar.copy(out=ohTs, in_=tp[:])
                        cand = psC.tile([BLOC, C], f32, tag="cand")
                        nc.tensor.matmul(cand[:], ohTs, tt_sb,
                                         start=True, stop=True,
                                         skip_group_check=True)
                        # hist add on DVE (cheaper than a second matmul)
                        nc.vector.tensor_tensor(
                            out=candsb, in0=cand[:],
                            in1=ring[pr][:, col * C:(col + 1) * C],
                            op=Alu.add)
                        nc.vector.tensor_reduce(out=mx32.ap(), in_=candsb,
                                                axis=AX, op=Alu.max)
                        nc.vector.max_index(
                            out=tg8[0:BLOC, 8 * (u - 1):8 * u],
                            in_max=mx8, in_values=candsb)
                        # cast, then masked copy: live lanes take the
                        # computed tag, dead lanes keep the prefilled last_tag
                        nc.vector.tensor_copy(
                            out=tgc,
                            in_=tg8[0:BLOC, 8 * (u - 1):8 * (u - 1) + 1])
                        nc.vector.copy_predicated(
                            out=tagf[:, u - 1:u],
                            mask=mk32_sb.ap()[0:BLOC, u:u + 1],
                            data=tgc)
                        ecol = s - 1 if s > 0 else CT - 1
                        epr = pr if s > 0 else 1 - pr
                        nc.vector.tensor_scalar(
                            out=hr[epr][:, ecol * C:(ecol + 1) * C],
                            in0=iotaf_sb, scalar1=tagf[:, u - 1:u],
                            scalar2=None, op0=Alu.is_equal)
                    nc.sync.dma_start(
                        out=out_d[:, cc * CT * C:(cc + 1) * CT * C],
                        in_=hr[pr])

    nc.compile()
    return nc


def _host_inputs(x, transitions, seq_lens, lanes):
    global _consts_cache
    if _consts_cache is None:
        _consts_cache = _build_consts()
    cst = _consts_cache
    tt, trep32, trep16 = _build_treps(transitions)
    tgrid = np.arange(T)[None, :]
    in_maps = []
    for c in range(lanes.shape[0]):
        lx = x[lanes[c]].reshape(BLOC, T * C).astype(np.float32)
        L = seq_lens[lanes[c]].astype(np.int64)
        mk = (tgrid < L[:, None]).astype(np.int8)          # [32, T]
        mk32 = np.ascontiguousarray(np.tile(mk, (4, 1)))    # [128, T]
        mk16 = np.ascontiguousarray(np.tile(mk[:16], (8, 1)))
        ivm = np.ascontiguousarray((tgrid >= L[:, None]).astype(np.int8))
        in_maps.append({
            "x": np.ascontiguousarray(lx),
            "trep32": trep32, "trep16": trep16, "tt": tt,
            "ident": cst["ident"], "esel2": cst["esel2"], "e16": cst["e16"],
            "eJ32": cst["eJ32"], "eJ16": cst["eJ16"], "iotaf": cst["iotaf"],
            "mk32": mk32, "mk16": mk16, "ivm": ivm,
        })
    return in_maps


TRACE = False
LAST_RESULT = None


def kernel(x, transitions, seq_lens):
    global LAST_RESULT
    from concourse.bass_utils import run_bass_kernel_spmd

    x = np.asarray(x, dtype=np.float32)
    transitions = np.asarray(transitions, dtype=np.float32)
    seq_lens = np.asarray(seq_lens)
    lanes = _assign_lanes(seq_lens)
    modes = _modes(seq_lens, lanes)
    if modes not in _prog_cache:
        _prog_cache[modes] = build_program(modes)
    nc = _prog_cache[modes]
    in_maps = _host_inputs(x, transitions, seq_lens, lanes)
    res = run_bass_kernel_spmd(nc, in_maps, list(range(NCORES)), trace=TRACE)
    LAST_RESULT = res
    out = np.empty((B, T, C), np.float32)
    for c in range(NCORES):
        out[lanes[c]] = res.results[c]["out"].reshape(BLOC, T, C)
    return out
